# revision 1
# baseline (speedup 1.0000x reference)
"""Trainium2 Bass kernel for nn_Attention_2284922602161 (linear attention).

Math per batch element b (C=512, Cq=64, N=4096):
    Q = Wq@x + bq            [Cq, N]
    K = Wk@x + bk            [Cq, N]
    V = Wv@x + bv            [C, N]
    Qn = Q / ||Q||_col ; Kn = K / ||K||_col      (L2 over channel dim per position)
    ksum = sum_n Kn + eps    [Cq]
    tailor[n] = 1 / (N + Qn[:,n].ksum)
    M1 = Kn @ V^T            [Cq, C]
    out[:,n] = gamma * tailor[n] * (V.sum(-1) + M1^T @ Qn[:,n])

Sharding: pure data-parallel, one batch element per NeuronCore (B=8 over 8 cores).

On-chip strategy (per core):
  - pass A: x tiles [c'=128, n=128] are the stationary matmul operand; moving
    operand is Wall^T = [Wv^T | Wq^T | Wk^T] [c', 640] so PSUM holds the
    TRANSPOSED projections [n, 640] (positions on partitions) where per-position
    L2 norms are free-dim reductions. x is host-relaid to partition-major and
    streamed in several large partition-contiguous DMAs.
  - phase 2 accumulates over all n tiles with RAW (bias-free) V:
    lhsT = [Kn^T | 1] [n=128, 65], rhs = V_raw^T tile [n=128, 512]
    -> psum [65, 512] = [M1_raw ; vsum_raw]; rhs = ones [128, 2] -> [65, 2]
    = [ksum ; N].  The V bias folds in afterwards as a rank-1 update:
    M1_aug += [ksum; N] (x) bv   (one K=1 fp32 matmul into the same psum).
  - pass C/D: per-position scalars (1/||q||, taylor denominator, gamma) are
    per-partition ops in the transposed layout, batched across all 32 tiles,
    folded into qs_aug = [s*Q^T | gamma*tailor] [128, 65], PE-transposed.
    ksum is replicated across partitions via a tiny fp32 PE outer product.
  - pass D/E interleaved per 512-column group: transpose 4 qs tiles, then one
    matmul per c-tile: out [c=128, n=512] = M1_aug_slice.T @ qs_aug; output
    rows stream out per half-row DMA.
Heavy matmuls run in float32r (1 cyc/row at N>=256, ~1.5e-4 rel err).
"""

import numpy as np

B, C, H, W = 8, 512, 64, 64
N = H * W              # 4096
CQ = 64
P = 128
NT = N // P            # 32 n-tiles
KT = C // P            # 4 contraction tiles
WALL = C + 2 * CQ      # 640 = [WvT | WqT | WkT]
EPS = 1e-6
NCHUNK = 512
XCHUNKS = (1, 3, 4, 8, 8, 8)   # n-tiles per x DMA (small first: PE starts early)

_CACHE = {}


def _build():
    import concourse.bacc as bacc
    import concourse.mybir as mybir
    import concourse.tile as tile
    import concourse.bass as bass
    from contextlib import ExitStack

    f32 = mybir.dt.float32
    f32r = mybir.dt.float32r

    nc = bacc.Bacc("TRN2", target_bir_lowering=False, debug=False,
                   enable_asserts=True, num_devices=8)

    # x host-relaid to [P, NT, KT*128]: partition-major, contiguous per row.
    x_d = nc.declare_dram_parameter("x", [P, NT, C], f32r, isOutput=False)
    w_d = nc.declare_dram_parameter("wall", [C, WALL], f32r, isOutput=False)
    b_d = nc.declare_dram_parameter("brep", [P, WALL], f32, isOutput=False)
    g_d = nc.declare_dram_parameter("gamma", [P, 1], f32, isOutput=False)
    id_d = nc.declare_dram_parameter("ident", [P, P], f32r, isOutput=False)
    on_d = nc.declare_dram_parameter("ones", [P, 2], f32r, isOutput=False)
    o32_d = nc.declare_dram_parameter("ones32", [P, NT], f32r, isOutput=False)
    orow_d = nc.declare_dram_parameter("onesrow", [1, P], f32, isOutput=False)
    out_d = nc.declare_dram_parameter("out", [C, N], f32, isOutput=True)

    with tile.TileContext(nc) as tc:
        with ExitStack() as ctx:
            const = ctx.enter_context(tc.tile_pool(name="const", bufs=1))
            xpool = ctx.enter_context(tc.tile_pool(name="xpool", bufs=2))
            vpool = ctx.enter_context(tc.tile_pool(name="vpool", bufs=6))
            scpool = ctx.enter_context(tc.tile_pool(name="scpool", bufs=2))
            obufp = ctx.enter_context(tc.tile_pool(name="obufp", bufs=1))

            # --- first x chunk, then weights (first MM needs x0 + w[k=0]) ---
            x_ap0 = x_d.ap()
            x_first = xpool.tile([P, XCHUNKS[0], KT, P], f32r,
                                 name="x_0", tag="x")
            nc.sync.dma_start(
                out=x_first,
                in_=x_ap0[:, 0:XCHUNKS[0], :].rearrange(
                    "p g (k n) -> p g k n", k=KT))
            w_sb = const.tile([P, KT, WALL], f32r)
            w_re = w_d.ap().rearrange("(k p) w -> p k w", p=P)
            for k in range(KT):
                nc.sync.dma_start(out=w_sb[:, k], in_=w_re[:, k])
            # small consts on the Pool queue (SWDGE) to keep SP free for x/out
            bias_rep = const.tile([P, WALL], f32)
            nc.gpsimd.dma_start(out=bias_rep, in_=b_d.ap())
            gamma_sb = const.tile([P, 1], f32)
            nc.gpsimd.dma_start(out=gamma_sb, in_=g_d.ap())
            ident_sb = const.tile([P, P], f32r)
            nc.gpsimd.dma_start(out=ident_sb, in_=id_d.ap())
            ones2_sb = const.tile([P, 2], f32r)
            nc.gpsimd.dma_start(out=ones2_sb, in_=on_d.ap())
            ones32_sb = const.tile([P, NT], f32r)
            nc.gpsimd.dma_start(out=ones32_sb, in_=o32_d.ap())
            onesrow_sb = const.tile([1, P], f32)
            nc.gpsimd.dma_start(out=onesrow_sb, in_=orow_d.ap())

            # --- persistent per-batch buffers ---
            qk_all = const.tile([P, NT, 2 * CQ], f32)   # raw (biased) [Q^T|K^T]
            kn_all = const.tile([P, NT, CQ + 1], f32r)  # [Kn^T | 1]
            qs_all = const.tile([P, NT, CQ + 1], f32r)  # [s*Q^T | gamma*tailor]
            q_ss = const.tile([P, NT], f32)
            qd = const.tile([P, NT], f32)
            qprod = const.tile([P, NT, CQ], f32)        # batched qdot product
            qf_all = const.tile([CQ + 1, N], f32r)      # transposed qs
            m1_sb = const.tile([CQ + 1, C], f32r)
            ks_eps = const.tile([P, CQ], f32)

            # aug ones column for all kn tiles in one strided write
            nc.vector.tensor_copy(out=kn_all[:, :, CQ], in_=ones32_sb)

            x_ap = x_d.ap()

            with tc.tile_pool(name="psM", bufs=1, space="PSUM") as psM_pool:
              with tc.tile_pool(name="psA", bufs=3, space="PSUM") as psA_pool:
                m1_ps = psM_pool.tile([CQ + 1, C], f32)
                ks_ps = psM_pool.tile([CQ + 1, 2], f32)

                pending = []
                vts = {}
                x_g = None
                x_base = 0
                xc = 0

                for i in range(NT):
                    if sum(XCHUNKS[:xc]) == i:
                        nt_chunk = XCHUNKS[xc]
                        if xc == 0:
                            x_g = x_first
                        else:
                            x_g = xpool.tile([P, nt_chunk, KT, P], f32r,
                                             name=f"x_{xc}", tag="x")
                            nc.sync.dma_start(
                                out=x_g,
                                in_=x_ap[:, i:i + nt_chunk, :].rearrange(
                                    "p g (k n) -> p g k n", k=KT))
                        x_base = i
                        xc += 1
                    x_t = x_g[:, i - x_base]

                    psA0 = psA_pool.tile([P, 320], f32, name=f"psA0_{i}",
                                         tag="psA0")
                    psA1 = psA_pool.tile([P, 320], f32, name=f"psA1_{i}",
                                         tag="psA1")
                    for k in range(KT):
                        nc.tensor.matmul(psA0, x_t[:, k, :], w_sb[:, k, 0:320],
                                         start=(k == 0), stop=(k == KT - 1))
                        nc.tensor.matmul(psA1, x_t[:, k, :], w_sb[:, k, 320:640],
                                         start=(k == 0), stop=(k == KT - 1))

                    # phase-2 matmuls, two tiles behind (pipeline slack)
                    if len(pending) >= 3:
                        j = pending.pop(0)
                        nc.tensor.matmul(m1_ps, kn_all[:, j, :], vts.pop(j),
                                         start=(j == 0), stop=False)
                        nc.tensor.matmul(ks_ps, kn_all[:, j, :], ones2_sb,
                                         start=(j == 0), stop=False)

                    # psum eviction: V raw (bias folds into M1 later)
                    vt = vpool.tile([P, C], f32r, name=f"vt_{i}", tag="vt")
                    vts[i] = vt
                    nc.vector.tensor_copy(out=vt[:, 0:320], in_=psA0)
                    nc.scalar.copy(out=vt[:, 320:512], in_=psA1[:, 0:192])
                    nc.vector.tensor_add(out=qk_all[:, i, :],
                                         in0=psA1[:, 192:320],
                                         in1=bias_rep[:, 512:640])
                    kT = qk_all[:, i, CQ:2 * CQ]

                    sck = scpool.tile([P, CQ], f32, name=f"sck_{i}", tag="sck")
                    kss = scpool.tile([P, 1], f32, name=f"kss_{i}", tag="kss")
                    nc.scalar.activation(
                        out=sck, in_=kT,
                        func=mybir.ActivationFunctionType.Square,
                        accum_out=kss)
                    krt = scpool.tile([P, 1], f32, name=f"krt_{i}", tag="krt")
                    nc.scalar.activation(out=krt, in_=kss,
                                         func=mybir.ActivationFunctionType.Sqrt)
                    krs = scpool.tile([P, 1], f32, name=f"krs_{i}", tag="krs")
                    nc.vector.reciprocal(out=krs, in_=krt)
                    nc.vector.tensor_scalar_mul(out=kn_all[:, i, 0:CQ], in0=kT,
                                                scalar1=krs)
                    pending.append(i)
                    if i in (NT // 2 - 1, NT - 1):
                        h0 = 0 if i < NT // 2 else NT // 2
                        h1 = h0 + NT // 2
                        nc.scalar.activation(
                            out=qprod[:, h0:h1, :],
                            in_=qk_all[:, h0:h1, 0:CQ],
                            func=mybir.ActivationFunctionType.Square)
                        qsh = q_ss[:, h0:h1]
                        qss3 = bass.AP(tensor=qsh.tensor, offset=qsh.offset,
                                       ap=[qsh.ap[0], qsh.ap[1], [1, 1]])
                        nc.vector.reduce_sum(out=qss3, in_=qprod[:, h0:h1, :],
                                             axis=mybir.AxisListType.X)

                # drain remaining phase-2 accumulation (M1 open for bias fold)
                for idx, j in enumerate(pending):
                    nc.tensor.matmul(m1_ps, kn_all[:, j, :], vts.pop(j),
                                     start=False, stop=False)
                    nc.tensor.matmul(ks_ps, kn_all[:, j, :], ones2_sb,
                                     start=False, stop=(idx == len(pending) - 1))

              # --- pass B (psA closed; psB banks now free) ---
              if True:
                qrt_all = const.tile([P, NT], f32)
                nc.scalar.activation(out=qrt_all, in_=q_ss,
                                     func=mybir.ActivationFunctionType.Sqrt)
                qrs_all = const.tile([P, NT], f32)
                nc.vector.reciprocal(out=qrs_all, in_=qrt_all)

                # ks column [ksum; N] -> row [1, 65]; then
                #  1) rank-1 V-bias fold:  M1_aug += ks_row^T (x) bv_row
                #  2) partition-replicate ksum via ones-row outer product
                ks_sb = const.tile([CQ + 1, 1], f32)
                nc.vector.tensor_copy(out=ks_sb, in_=ks_ps[0:CQ + 1, 0:1])
                with tc.tile_pool(name="psB", bufs=1, space="PSUM") as psB_pool:
                    ksr_ps = psB_pool.tile([1, CQ + 1], f32)
                    nc.tensor.transpose(
                        ksr_ps, ks_sb,
                        ident_sb.bitcast(f32)[0:CQ + 1, 0:CQ + 1])
                    ksrow = const.tile([1, CQ + 1], f32)
                    nc.vector.tensor_copy(out=ksrow, in_=ksr_ps)
                    # V-bias fold into the still-open M1 psum accumulation
                    nc.tensor.matmul(m1_ps, ksrow, bias_rep[0:1, 0:C],
                                     start=False, stop=True)
                    nc.vector.tensor_copy(out=m1_sb, in_=m1_ps)
                    # ksum replicated to all partitions (+eps)
                    rep_ps = psB_pool.tile([P, CQ], f32)
                    nc.tensor.matmul(rep_ps, onesrow_sb, ksrow[0:1, 0:CQ],
                                     start=True, stop=True)
                    nc.vector.tensor_scalar_add(out=ks_eps, in0=rep_ps,
                                                scalar1=EPS)

            # --- pass C: qdot + per-position scalars (batched) ---
            ksb_ap = bass.AP(tensor=ks_eps.tensor, offset=ks_eps.offset,
                             ap=[ks_eps.ap[0], [0, NT], [1, CQ]])
            nc.vector.tensor_mul(out=qprod, in0=qk_all[:, :, 0:CQ], in1=ksb_ap)
            qd3 = bass.AP(tensor=qd.tensor, offset=qd.offset,
                          ap=[qd.ap[0], qd.ap[1], [1, 1]])
            nc.vector.reduce_sum(out=qd3, in_=qprod, axis=mybir.AxisListType.X)
            dn = const.tile([P, NT], f32)
            nc.vector.tensor_mul(out=dn, in0=qd, in1=qrs_all)
            nc.vector.tensor_scalar_add(out=dn, in0=dn, scalar1=float(N))
            tailor = const.tile([P, NT], f32)
            nc.vector.reciprocal(out=tailor, in_=dn)
            aug_all = const.tile([P, NT], f32)
            nc.vector.tensor_scalar_mul(out=aug_all, in0=tailor, scalar1=gamma_sb)
            s_all = const.tile([P, NT], f32)
            nc.vector.tensor_mul(out=s_all, in0=aug_all, in1=qrs_all)

            # qs: aug columns batched; scaled Q written per group below
            nc.vector.tensor_copy(out=qs_all[:, :, CQ], in_=aug_all)

            # --- pass D+E interleaved per 512-col group ---
            obufs = [obufp.tile([P, N], f32, name=f"obuf_{c}", tag=f"obuf{c}")
                     for c in range(KT)]
            NG = NCHUNK // P  # 4 tiles per group
            with tc.tile_pool(name="psT", bufs=2, space="PSUM") as psT_pool, \
                 tc.tile_pool(name="psE", bufs=3, space="PSUM") as psE_pool:
                s_b = bass.AP(tensor=s_all.tensor, offset=s_all.offset,
                              ap=[s_all.ap[0], s_all.ap[1], [0, CQ]])
                for g in range(NT // NG):
                    s_sl = s_all[:, g * NG:(g + 1) * NG]
                    nc.vector.tensor_mul(
                        out=qs_all[:, g * NG:(g + 1) * NG, 0:CQ],
                        in0=qk_all[:, g * NG:(g + 1) * NG, 0:CQ],
                        in1=bass.AP(tensor=s_sl.tensor, offset=s_sl.offset,
                                    ap=[s_sl.ap[0], s_sl.ap[1], [0, CQ]]))
                    tr_ps = psT_pool.tile([CQ + 1, NG, P], f32r,
                                          name=f"tr_{g}", tag="tr")
                    for u in range(NG):
                        i = g * NG + u
                        nc.tensor.transpose(tr_ps[:, u, :], qs_all[:, i, :],
                                            ident_sb)
                    nc.vector.tensor_copy(
                        out=qf_all[:, g * NCHUNK:(g + 1) * NCHUNK],
                        in_=tr_ps.rearrange("m u n -> m (u n)"))
                    for c in range(KT):
                        out_ps = psE_pool.tile([P, NCHUNK], f32,
                                               name=f"ops_{c}_{g}", tag="ops")
                        nc.tensor.matmul(
                            out_ps, m1_sb[:, c * P:(c + 1) * P],
                            qf_all[:, g * NCHUNK:(g + 1) * NCHUNK],
                            start=True, stop=True)
                        dst = obufs[c][:, g * NCHUNK:(g + 1) * NCHUNK]
                        if c == 3:
                            nc.scalar.copy(out=dst, in_=out_ps)
                        else:
                            nc.vector.tensor_copy(out=dst, in_=out_ps)
                        nc.sync.dma_start(
                            out=out_d.ap()[c * P:(c + 1) * P,
                                           g * NCHUNK:(g + 1) * NCHUNK],
                            in_=dst)

    nc.compile()
    return nc


def _get_nc():
    if "nc" not in _CACHE:
        _CACHE["nc"] = _build()
    return _CACHE["nc"]


def _prep_inputs(x, Wq, bq, Wk, bk, Wv, bv, gamma):
    x = np.ascontiguousarray(np.asarray(x, dtype=np.float32)).reshape(B, C, N)
    # relayout: x[b, k*128+p, i*128+j] -> xh[b, p, i, k*128+j]
    xh = np.ascontiguousarray(
        x.reshape(B, KT, P, NT, P).transpose(0, 2, 3, 1, 4).reshape(B, P, NT, C))
    wall = np.ascontiguousarray(np.concatenate(
        [np.asarray(Wv, np.float32).T,
         np.asarray(Wq, np.float32).T,
         np.asarray(Wk, np.float32).T], axis=1))
    ball = np.concatenate(
        [np.asarray(bv, np.float32),
         np.asarray(bq, np.float32),
         np.asarray(bk, np.float32)])
    brep = np.ascontiguousarray(np.tile(ball[None, :], (P, 1)))
    gam = np.full((P, 1), np.float32(np.asarray(gamma).reshape(-1)[0]),
                  dtype=np.float32)
    return {
        "x": xh,
        "wall": wall,
        "brep": brep,
        "gamma": gam,
        "ident": np.eye(P, dtype=np.float32),
        "ones": np.ones((P, 2), dtype=np.float32),
        "ones32": np.ones((P, NT), dtype=np.float32),
        "onesrow": np.ones((1, P), dtype=np.float32),
    }


def kernel(x, Wq, bq, Wk, bk, Wv, bv, gamma, _trace=False):
    from concourse.bass_utils import run_bass_kernel_spmd

    common = _prep_inputs(x, Wq, bq, Wk, bk, Wv, bv, gamma)
    xh = common.pop("x")
    nc = _get_nc()
    in_maps = [{"x": xh[i], **common} for i in range(B)]
    res = run_bass_kernel_spmd(nc, in_maps, list(range(B)), trace=_trace)
    out = np.stack([res.results[i]["out"] for i in range(B)])
    if _trace:
        _CACHE["last_results"] = res
    return out.reshape(B, C, H, W).astype(np.float32)



# revision 49
# speedup vs baseline: 1.1706x; 1.1706x over previous
"""Trainium2 Bass kernel for nn_Attention_2284922602161 (linear attention).

Math per batch element b (C=512, Cq=64, N=4096):
    Q = Wq@x + bq            [Cq, N]
    K = Wk@x + bk            [Cq, N]
    V = Wv@x + bv            [C, N]
    Qn = Q / ||Q||_col ; Kn = K / ||K||_col      (L2 over channel dim per position)
    ksum = sum_n Kn + eps    [Cq]
    tailor[n] = 1 / (N + Qn[:,n].ksum)
    M1 = Kn @ V^T            [Cq, C]
    out[:,n] = gamma * tailor[n] * (V.sum(-1) + M1^T @ Qn[:,n])

Sharding: pure data-parallel, one batch element per NeuronCore (B=8 over 8 cores).

v8 dataflow — V never materialized, x loaded ONCE (bf16):
    M1_aug^T = A_augT @ ... with A_augT[c,m] accumulated directly as four
    [128,65] PSUM accumulators: per n-tile, PE transposes the four xa
    sub-tiles (bf16, 53ns each) to get x^T on positions-partitions, then
    A_T[k] += xt[:, k-slice]^T_stationary @ [Kn^T|1].
    M1_aug = (A_augT-slices)^T @ WvT + [ksum;N] (x) bv.
    Q,K biases fold into the projection matmul as a rank-1 ones-row
    update, so PSUM evictions are plain copies and Act reads PSUM.
    Everything that depends only on Q/K (ksum row via ones^T @ kn_red,
    tailor scalars, qs scaling, output transposes + qf evictions) runs
    while the tail of x still streams; after the last A matmul only
    at->M1->m1_sb gates the out pass. Output bf16, one DMA per
    512-column group, PSUM evicted in [P,1024] pairs on Act/DVE.
"""

import numpy as np

B, C, H, W = 8, 512, 64, 64
N = H * W              # 4096
CQ = 64
P = 128
NT = N // P            # 32 n-tiles
KT = C // P            # 4 contraction tiles
QK = 2 * CQ            # 128 projected q|k channels
EPS = 1e-6
NCHUNK = 512
XCHUNKS = (4, 4, 8, 8, 8)
ALAG = 3               # A-matmul pipeline lag behind QK/transpose
_CACHE = {}


def _build():
    import concourse.bacc as bacc
    import concourse.mybir as mybir
    import concourse.tile as tile
    import concourse.bass as bass
    from contextlib import ExitStack

    f32 = mybir.dt.float32
    f32r = mybir.dt.float32r
    bf16 = mybir.dt.bfloat16

    nc = bacc.Bacc("TRN2", target_bir_lowering=False, debug=False,
                   enable_asserts=True, num_devices=8)

    xa_d = nc.declare_dram_parameter("xa", [P, NT, C], bf16, isOutput=False)
    wqk_d = nc.declare_dram_parameter("wqk", [C, QK], bf16, isOutput=False)
    # consts split by NATIVE dtype — fp32r matmul operands must be written
    # as f32r and bf16 matmul operands as bf16 (bitcasts are rejected)
    idb_d = nc.declare_dram_parameter("identb", [P, P + NT], bf16,
                                      isOutput=False)  # identb | ones32
    cprb_d = nc.declare_dram_parameter("cprb", [1, 2 * P], bf16,
                                       isOutput=False)  # brow | ones1p
    ckr_d = nc.declare_dram_parameter("ckr", [P, 327], f32r,
                                      isOutput=False)  # identf|onescol|knred0
    cprr_d = nc.declare_dram_parameter("cprr", [1, 640], f32r,
                                       isOutput=False)  # bvrow | onesrow
    wvt_d = nc.declare_dram_parameter("wvt", [P, KT * C], bf16,
                                      isOutput=False)
    out_d = nc.declare_dram_parameter("out", [C, N], bf16, isOutput=True)

    with tile.TileContext(nc) as tc:
        with ExitStack() as ctx:
            const = ctx.enter_context(tc.tile_pool(name="const", bufs=1))
            xapool = ctx.enter_context(tc.tile_pool(name="xapool", bufs=3))
            xtpool = ctx.enter_context(tc.tile_pool(name="xtpool", bufs=6))
            scpool = ctx.enter_context(tc.tile_pool(name="scpool", bufs=3))
            obufp = ctx.enter_context(tc.tile_pool(name="obufp", bufs=3))

            xa_ap = xa_d.ap()
            wqk_sb = const.tile([P, KT, QK], bf16)
            nc.sync.dma_start(
                out=wqk_sb, in_=wqk_d.ap().rearrange("(k p) w -> p k w", p=P))
            # consts on the Pool queue (SWDGE): small/early ones first
            idb_sb = const.tile([P, P + NT], bf16)
            nc.gpsimd.dma_start(out=idb_sb, in_=idb_d.ap())
            identb_sb = idb_sb[:, 0:P]
            ones32_sb = idb_sb[:, P:P + NT]
            cprb_sb = const.tile([1, 2 * P], bf16)
            nc.gpsimd.dma_start(out=cprb_sb, in_=cprb_d.ap())
            brow_sb = cprb_sb[0:1, 0:P]
            ones1p_sb = cprb_sb[0:1, P:2 * P]
            ckr_sb = const.tile([P, 327], f32r)
            nc.gpsimd.dma_start(out=ckr_sb, in_=ckr_d.ap())
            identf_sb = ckr_sb[:, 0:128]
            onescol_sb = ckr_sb[:, 128:129]
            cprr_sb = const.tile([1, 640], f32r)
            nc.gpsimd.dma_start(out=cprr_sb, in_=cprr_d.ap())
            bvrow_sb = cprr_sb[0:1, 0:512]
            onesrow_sb = cprr_sb[0:1, 512:640]
            wvt_sb = const.tile([P, KT, C], bf16)
            nc.gpsimd.dma_start(
                out=wvt_sb,
                in_=wvt_d.ap().rearrange("p (k c) -> p k c", k=KT))

            # --- persistent per-batch buffers ---
            q_all = const.tile([P, NT, CQ], bf16)       # biased Q^T
            kn_all = const.tile([P, NT, CQ + 1], bf16)  # [Kn^T | 1]
            # 66-wide slices (fp32r matmul moving size must be even);
            # carved from the DMA-zeroed f32r const so pad columns are 0
            kn_red = ckr_sb[:, 129:327].rearrange("p (h m) -> p h m", h=3)
            qs_all = const.tile([P, NT, CQ + 1], f32r)  # [s*Q^T | gamma*tailor]
            q_ss = const.tile([P, NT], f32)
            qd = const.tile([P, NT], f32)
            qsq = const.tile([P, NT, CQ], bf16)         # scratch squares/prods
            qf_all = const.tile([CQ + 1, N], f32r)      # transposed qs
            at_sb = const.tile([P, KT, CQ + 1], bf16)   # A_aug^T slices
            m1_sb = const.tile([CQ + 1, C], f32r)
            ks_eps = const.tile([P, CQ], f32)
            ksrow = const.tile([1, CQ + 1], f32r)
            M = CQ + 1

            # aug ones column for all kn tiles in one strided write
            nc.vector.tensor_copy(out=kn_all[:, :, CQ], in_=ones32_sb)
            # preload activation tables (Square/Sqrt) while DMAs stream
            warm = const.tile([1, 1], f32)
            nc.scalar.activation(out=warm, in_=wqk_sb[0:1, 0, 0:1],
                                 func=mybir.ActivationFunctionType.Square)
            nc.scalar.activation(out=warm, in_=warm,
                                 func=mybir.ActivationFunctionType.Sqrt)

            with tc.tile_pool(name="psQK", bufs=4, space="PSUM") as psQKp, \
                 tc.tile_pool(name="psXT", bufs=3, space="PSUM") as psXTp, \
                 tc.tile_pool(name="psA", bufs=1, space="PSUM") as psAp:
                at_ps = psAp.tile([P, KT, M], f32)

                # ---- loop 1: stream x; QK projection + x^T + kn + A ------
                pending = []
                xts = {}
                xa_g = None
                x_base = 0
                xc = 0
                for i in range(NT):
                    if sum(XCHUNKS[:xc]) == i:
                        g = XCHUNKS[xc]
                        xa_g = xapool.tile([P, g, KT, P], bf16,
                                           name=f"xa_{xc}", tag="xa")
                        nc.sync.dma_start(
                            out=xa_g,
                            in_=xa_ap[:, i:i + g, :].rearrange(
                                "p g (k n) -> p g k n", k=KT))
                        x_base = i
                        xc += 1
                    xa_t = xa_g[:, i - x_base]

                    psqk = psQKp.tile([P, QK], f32, name=f"qk_{i}", tag="qk")
                    xt_ps = psXTp.tile([P, KT, P], bf16, name=f"xt_{i}",
                                       tag="xt")
                    for k in range(KT):
                        nc.tensor.matmul(psqk, xa_t[:, k, :], wqk_sb[:, k, :],
                                         start=(k == 0), stop=False)
                    # fold Q,K biases in as rank-1 ones^T (x) [bq|bk]
                    nc.tensor.matmul(psqk, ones1p_sb, brow_sb,
                                     start=False, stop=True)
                    # x^T tile via PE transposes (bf16)
                    for k in range(KT):
                        nc.tensor.transpose(xt_ps[:, k, :], xa_t[:, k, :],
                                            identb_sb)
                    xt = xtpool.tile([P, KT, P], bf16, name=f"xts_{i}",
                                     tag="xts")
                    xts[i] = xt
                    nc.vector.tensor_copy(out=xt, in_=xt_ps)

                    # A^T accumulation, ALAG tiles behind
                    if len(pending) >= ALAG:
                        j = pending.pop(0)
                        xtj = xts.pop(j)
                        for k in range(KT):
                            # start zeroes the WHOLE bank: only the very
                            # first matmul into at_ps may set it
                            nc.tensor.matmul(at_ps[:, k, :], xtj[:, k, :],
                                             kn_all[:, j, :],
                                             start=(j == 0 and k == 0),
                                             stop=False,
                                             skip_group_check=True)

                    # K chain: kss -> 1/||K|| -> kn (bf16), straight off PSUM
                    sck = scpool.tile([P, CQ], f32, name=f"sck_{i}", tag="sck")
                    kss = scpool.tile([P, 1], f32, name=f"kss_{i}", tag="kss")
                    nc.scalar.activation(
                        out=sck, in_=psqk[:, CQ:QK],
                        func=mybir.ActivationFunctionType.Square,
                        accum_out=kss)
                    krt = scpool.tile([P, 1], f32, name=f"krt_{i}", tag="krt")
                    nc.scalar.activation(out=krt, in_=kss,
                                         func=mybir.ActivationFunctionType.Sqrt)
                    krs = scpool.tile([P, 1], f32, name=f"krs_{i}", tag="krs")
                    nc.vector.reciprocal(out=krs, in_=krt)
                    # GPSIMD cannot read PSUM: kn scale on DVE, Q copy on Act
                    nc.vector.tensor_scalar_mul(out=kn_all[:, i, 0:CQ],
                                                in0=psqk[:, CQ:QK],
                                                scalar1=krs)
                    nc.scalar.copy(out=q_all[:, i, :], in_=psqk[:, 0:CQ])
                    pending.append(i)
                    # batched q squares + kn partial sums: 0..15 at i=15,
                    # 16..30 at i=30, tile 31 separately (so the mid-phase
                    # barrier chain only waits on tile 31's own tiny ops)
                    if i in (NT // 2 - 1, NT - 2):
                        h = 0 if i < NT // 2 else 1
                        h0 = h * (NT // 2)
                        h1 = (h + 1) * (NT // 2) - h
                        with nc.allow_low_precision(reason="q squares bf16"):
                            nc.vector.tensor_mul(out=qsq[:, h0:h1, :],
                                                 in0=q_all[:, h0:h1, :],
                                                 in1=q_all[:, h0:h1, :])
                        qsh = q_ss[:, h0:h1]
                        qss3 = bass.AP(tensor=qsh.tensor, offset=qsh.offset,
                                       ap=[qsh.ap[0], qsh.ap[1], [1, 1]])
                        nc.vector.reduce_sum(out=qss3, in_=qsq[:, h0:h1, :],
                                             axis=mybir.AxisListType.X)
                        ksl = kn_all[:, h0:h1, :]
                        ksw = bass.AP(tensor=ksl.tensor, offset=ksl.offset,
                                      ap=[ksl.ap[0], ksl.ap[2], ksl.ap[1]])
                        krl = kn_red[:, h, 0:CQ + 1]
                        kr3 = bass.AP(tensor=krl.tensor, offset=krl.offset,
                                      ap=[krl.ap[0], krl.ap[1], [1, 1]])
                        with nc.allow_low_precision(reason="f32r ksum"):
                            nc.vector.reduce_sum(out=kr3, in_=ksw,
                                                 axis=mybir.AxisListType.X)
                    if i == NT - 1:
                        nc.vector.tensor_copy(out=kn_red[:, 2, 0:CQ + 1],
                                              in_=kn_all[:, i, :])
                        scq = scpool.tile([P, CQ], bf16, name="scq31",
                                          tag="scq")
                        with nc.allow_low_precision(reason="scratch sq"):
                            nc.scalar.activation(
                                out=scq, in_=q_all[:, i, :],
                                func=mybir.ActivationFunctionType.Square,
                                accum_out=q_ss[:, i:i + 1])

                # drain A^T accumulation
                for idx, j in enumerate(pending):
                    xtj = xts.pop(j)
                    last = idx == len(pending) - 1
                    for k in range(KT):
                        nc.tensor.matmul(at_ps[:, k, :], xtj[:, k, :],
                                         kn_all[:, j, :],
                                         start=False, stop=last,
                                         skip_group_check=True)
                # A_aug^T -> bf16 sbuf (Act; one strided op)
                nc.scalar.copy(out=at_sb, in_=at_ps)

            with tc.tile_pool(name="psB", bufs=1, space="PSUM") as psB, \
                 tc.tile_pool(name="psT", bufs=2, space="PSUM") as psT_pool:
                # [ksum; N] row: ones^T @ kn_red (3 slices), add slices
                MP = CQ + 2
                ksr_ps = psB.tile([1, 3 * MP], f32)
                nc.tensor.matmul(
                    ksr_ps, onescol_sb,
                    kn_red.rearrange("p h m -> p (h m)"),
                    start=True, stop=True)
                ksr_sb = const.tile([1, 3 * MP], f32r)
                nc.vector.tensor_copy(out=ksr_sb, in_=ksr_ps)
                ksrow2 = const.tile([1, M], f32r)
                nc.vector.tensor_add(out=ksrow2, in0=ksr_sb[0:1, 0:M],
                                     in1=ksr_sb[0:1, MP:MP + M])
                nc.vector.tensor_add(out=ksrow, in0=ksrow2,
                                     in1=ksr_sb[0:1, 2 * MP:2 * MP + M])
                rep_ps = psB.tile([P, CQ], f32)
                nc.tensor.matmul(rep_ps, onesrow_sb, ksrow[0:1, 0:CQ],
                                 start=True, stop=True)
                nc.vector.tensor_scalar_add(out=ks_eps, in0=rep_ps,
                                            scalar1=EPS)
                ksb = const.tile([P, CQ], bf16)
                nc.vector.tensor_copy(out=ksb, in_=ks_eps)

                # M1 = A_aug^T-slices @ WvT + [ksum;N] (x) bv
                m1_ps = psB.tile([M, C], f32)
                for k in range(KT):
                    nc.tensor.matmul(m1_ps, at_sb[:, k, :], wvt_sb[:, k, :],
                                     start=(k == 0), stop=False)
                nc.tensor.matmul(m1_ps, ksrow, bvrow_sb,
                                 start=False, stop=True)
                # evict on Act (GPSIMD cannot read PSUM; DVE busy with qd)
                nc.scalar.copy(out=m1_sb, in_=m1_ps)

                # tailor scalars
                qrt_all = const.tile([P, NT], f32)
                nc.scalar.activation(out=qrt_all, in_=q_ss,
                                     func=mybir.ActivationFunctionType.Sqrt)
                qrs_all = const.tile([P, NT], f32)
                nc.vector.reciprocal(out=qrs_all, in_=qrt_all)
                ksb_ap = bass.AP(tensor=ksb.tensor, offset=ksb.offset,
                                 ap=[ksb.ap[0], [0, NT], [1, CQ]])
                nc.vector.tensor_mul(out=qsq, in0=q_all, in1=ksb_ap)
                nc.vector.reduce_sum(out=qd, in_=qsq,
                                     axis=mybir.AxisListType.X)
                dn = const.tile([P, NT], f32)
                nc.vector.tensor_mul(out=dn, in0=qd, in1=qrs_all)
                nc.vector.tensor_scalar_add(out=dn, in0=dn, scalar1=float(N))
                tailor = const.tile([P, NT], f32)
                nc.vector.reciprocal(out=tailor, in_=dn)
                s_all = const.tile([P, NT], bf16)
                nc.vector.tensor_mul(out=s_all, in0=tailor, in1=qrs_all)
                nc.vector.tensor_copy(out=qs_all[:, :, CQ], in_=tailor)

                NG = NCHUNK // P  # 4 tiles per group
                NGR = NT // NG
                # qs scale muls, alternating DVE/Pool
                for g in range(NGR):
                    s_sl = s_all[:, g * NG:(g + 1) * NG]
                    eng = nc.vector if g % 2 == 0 else nc.gpsimd
                    eng.tensor_mul(
                        out=qs_all[:, g * NG:(g + 1) * NG, 0:CQ],
                        in0=q_all[:, g * NG:(g + 1) * NG, :],
                        in1=bass.AP(tensor=s_sl.tensor, offset=s_sl.offset,
                                    ap=[s_sl.ap[0], s_sl.ap[1], [0, CQ]]))
                # all transposes + qf evictions (Act/DVE alternate)
                for g in range(NGR):
                    tr_ps = psT_pool.tile([CQ + 1, NG, P], f32r,
                                          name=f"tr_{g}", tag="tr")
                    for u in range(NG):
                        nc.tensor.transpose(tr_ps[:, u, :],
                                            qs_all[:, g * NG + u, :],
                                            identf_sb)
                    dstq = qf_all[:, g * NCHUNK:(g + 1) * NCHUNK]
                    srcq = tr_ps.rearrange("m u n -> m (u n)")
                    if g % 2 == 0:
                        nc.scalar.copy(out=dstq, in_=srcq)
                    else:
                        nc.vector.tensor_copy(out=dstq, in_=srcq)

            # ---- out pass: matmuls, pair evictions, one DMA/group --------
            with tc.tile_pool(name="psE", bufs=3, space="PSUM") as psE_pool:
                for g in range(NT // NG):
                    dst = obufp.tile([P, KT, NCHUNK], bf16,
                                     name=f"ob_{g}", tag="ob")
                    for half in range(2):
                        out_ps = psE_pool.tile([P, 2, NCHUNK], f32,
                                               name=f"ops_{half}_{g}",
                                               tag="ops")
                        for ci in range(2):
                            c = half * 2 + ci
                            nc.tensor.matmul(
                                out_ps[:, ci, :],
                                m1_sb[:, c * P:(c + 1) * P],
                                qf_all[:, g * NCHUNK:(g + 1) * NCHUNK],
                                start=True, stop=True)
                        dsth = dst[:, half * 2:half * 2 + 2, :]
                        if half == 0:
                            nc.scalar.copy(out=dsth, in_=out_ps)
                        else:
                            nc.vector.tensor_copy(out=dsth, in_=out_ps)
                    nc.sync.dma_start(
                        out=out_d.ap()[:, g * NCHUNK:(g + 1) * NCHUNK]
                        .rearrange("(c p) n -> p c n", p=P),
                        in_=dst)

    nc.compile()
    return nc


def _get_nc():
    if "nc" not in _CACHE:
        _CACHE["nc"] = _build()
    return _CACHE["nc"]


def _prep_inputs(x, Wq, bq, Wk, bk, Wv, bv, gamma):
    import ml_dtypes
    bf = ml_dtypes.bfloat16
    x = np.ascontiguousarray(np.asarray(x, dtype=np.float32)).reshape(B, C, N)
    # XA[b, p, i, k*128+j] = x[b, k*128+p, i*128+j]
    xa = np.ascontiguousarray(
        x.reshape(B, KT, P, NT, P).transpose(0, 2, 3, 1, 4)
        .reshape(B, P, NT, C).astype(bf))
    wqk = np.ascontiguousarray(np.concatenate(
        [np.asarray(Wq, np.float32).T,
         np.asarray(Wk, np.float32).T], axis=1).astype(bf))
    # wvt[p, k, c] = gamma * Wv.T[k*128+p, c]  (gamma folded in)
    gscal = np.float32(np.asarray(gamma).reshape(-1)[0])
    wvt = np.ascontiguousarray(
        (np.asarray(Wv, np.float32).T * gscal).reshape(KT, P, C)
        .transpose(1, 0, 2).reshape(P, KT * C).astype(bf))
    ckr = np.concatenate([
        np.eye(P, dtype=np.float32),                             # identf
        np.ones((P, 1), dtype=np.float32),                       # onescol
        np.zeros((P, 198), dtype=np.float32),                    # kn_red init
    ], axis=1)
    cprb = np.concatenate([
        np.concatenate([np.asarray(bq, np.float32),
                        np.asarray(bk, np.float32)])[None, :],   # brow
        np.ones((1, P), dtype=np.float32),                       # ones1p
    ], axis=1).astype(bf)
    cprr = np.concatenate([
        (np.asarray(bv, np.float32) * gscal)[None, :],           # gamma*bv
        np.ones((1, P), dtype=np.float32),                       # onesrow
    ], axis=1)
    return {
        "xa": xa,
        "wqk": wqk,
        "identb": np.concatenate(
            [np.eye(P, dtype=np.float32),
             np.ones((P, NT), dtype=np.float32)], axis=1).astype(bf),
        "cprb": np.ascontiguousarray(cprb),
        "ckr": np.ascontiguousarray(ckr),
        "cprr": np.ascontiguousarray(cprr),
        "wvt": wvt,
    }


def kernel(x, Wq, bq, Wk, bk, Wv, bv, gamma, _trace=False):
    from concourse.bass_utils import run_bass_kernel_spmd

    common = _prep_inputs(x, Wq, bq, Wk, bk, Wv, bv, gamma)
    xa = common.pop("xa")
    nc = _get_nc()
    in_maps = [{"xa": xa[i], **common} for i in range(B)]
    res = run_bass_kernel_spmd(nc, in_maps, list(range(B)), trace=_trace)
    out = np.stack([np.asarray(res.results[i]["out"]).astype(np.float32)
                    for i in range(B)])
    if _trace:
        _CACHE["last_results"] = res
    return out.reshape(B, C, H, W)


# revision 50
# speedup vs baseline: 1.1869x; 1.0139x over previous
"""Trainium2 Bass kernel for nn_Attention_2284922602161 (linear attention).

Math per batch element b (C=512, Cq=64, N=4096):
    Q = Wq@x + bq            [Cq, N]
    K = Wk@x + bk            [Cq, N]
    V = Wv@x + bv            [C, N]
    Qn = Q / ||Q||_col ; Kn = K / ||K||_col      (L2 over channel dim per position)
    ksum = sum_n Kn + eps    [Cq]
    tailor[n] = 1 / (N + Qn[:,n].ksum)
    M1 = Kn @ V^T            [Cq, C]
    out[:,n] = gamma * tailor[n] * (V.sum(-1) + M1^T @ Qn[:,n])

Sharding: pure data-parallel, one batch element per NeuronCore (B=8 over 8 cores).

v8 dataflow — V never materialized, x loaded ONCE (bf16):
    M1_aug^T = A_augT @ ... with A_augT[c,m] accumulated directly as four
    [128,65] PSUM accumulators: per n-tile, PE transposes the four xa
    sub-tiles (bf16, 53ns each) to get x^T on positions-partitions, then
    A_T[k] += xt[:, k-slice]^T_stationary @ [Kn^T|1].
    M1_aug = (A_augT-slices)^T @ WvT + [ksum;N] (x) bv.
    Q,K biases fold into the projection matmul as a rank-1 ones-row
    update, so PSUM evictions are plain copies and Act reads PSUM.
    Everything that depends only on Q/K (ksum row via ones^T @ kn_red,
    tailor scalars, qs scaling, output transposes + qf evictions) runs
    while the tail of x still streams; after the last A matmul only
    at->M1->m1_sb gates the out pass. Output bf16, one DMA per
    512-column group, PSUM evicted in [P,1024] pairs on Act/DVE.
"""

import numpy as np

B, C, H, W = 8, 512, 64, 64
N = H * W              # 4096
CQ = 64
P = 128
NT = N // P            # 32 n-tiles
KT = C // P            # 4 contraction tiles
QK = 2 * CQ            # 128 projected q|k channels
EPS = 1e-6
NCHUNK = 512
XCHUNKS = (4, 4, 8, 8, 8)
ALAG = 3               # A-matmul pipeline lag behind QK/transpose
_CACHE = {}


def _build():
    import concourse.bacc as bacc
    import concourse.mybir as mybir
    import concourse.tile as tile
    import concourse.bass as bass
    from contextlib import ExitStack

    f32 = mybir.dt.float32
    f32r = mybir.dt.float32r
    bf16 = mybir.dt.bfloat16

    nc = bacc.Bacc("TRN2", target_bir_lowering=False, debug=False,
                   enable_asserts=True, num_devices=8)

    xa_d = nc.declare_dram_parameter("xa", [P, NT, C], bf16, isOutput=False)
    wqk_d = nc.declare_dram_parameter("wqk", [C, QK], bf16, isOutput=False)
    # consts split by NATIVE dtype — fp32r matmul operands must be written
    # as f32r and bf16 matmul operands as bf16 (bitcasts are rejected)
    idb_d = nc.declare_dram_parameter("identb", [P, P + NT], bf16,
                                      isOutput=False)  # identb | ones32
    cprb_d = nc.declare_dram_parameter("cprb", [1, 2 * P], bf16,
                                       isOutput=False)  # brow | ones1p
    ckr_d = nc.declare_dram_parameter("ckr", [P, 327], f32r,
                                      isOutput=False)  # identf|onescol|knred0
    cprr_d = nc.declare_dram_parameter("cprr", [1, 640], f32r,
                                       isOutput=False)  # bvrow | onesrow
    wvt_d = nc.declare_dram_parameter("wvt", [P, KT * C], bf16,
                                      isOutput=False)
    out_d = nc.declare_dram_parameter("out", [C, N], bf16, isOutput=True)

    with tile.TileContext(nc) as tc:
        with ExitStack() as ctx:
            const = ctx.enter_context(tc.tile_pool(name="const", bufs=1))
            xapool = ctx.enter_context(tc.tile_pool(name="xapool", bufs=3))
            xtpool = ctx.enter_context(tc.tile_pool(name="xtpool", bufs=6))
            scpool = ctx.enter_context(tc.tile_pool(name="scpool", bufs=3))
            obufp = ctx.enter_context(tc.tile_pool(name="obufp", bufs=3))

            xa_ap = xa_d.ap()
            wqk_sb = const.tile([P, KT, QK], bf16)
            nc.sync.dma_start(
                out=wqk_sb, in_=wqk_d.ap().rearrange("(k p) w -> p k w", p=P))
            # consts on the Pool queue (SWDGE): small/early ones first
            idb_sb = const.tile([P, P + NT], bf16)
            nc.gpsimd.dma_start(out=idb_sb, in_=idb_d.ap())
            identb_sb = idb_sb[:, 0:P]
            ones32_sb = idb_sb[:, P:P + NT]
            cprb_sb = const.tile([1, 2 * P], bf16)
            nc.gpsimd.dma_start(out=cprb_sb, in_=cprb_d.ap())
            brow_sb = cprb_sb[0:1, 0:P]
            ones1p_sb = cprb_sb[0:1, P:2 * P]
            ckr_sb = const.tile([P, 327], f32r)
            nc.gpsimd.dma_start(out=ckr_sb, in_=ckr_d.ap())
            identf_sb = ckr_sb[:, 0:128]
            onescol_sb = ckr_sb[:, 128:129]
            cprr_sb = const.tile([1, 640], f32r)
            nc.gpsimd.dma_start(out=cprr_sb, in_=cprr_d.ap())
            bvrow_sb = cprr_sb[0:1, 0:512]
            onesrow_sb = cprr_sb[0:1, 512:640]
            wvt_sb = const.tile([P, KT, C], bf16)
            nc.gpsimd.dma_start(
                out=wvt_sb,
                in_=wvt_d.ap().rearrange("p (k c) -> p k c", k=KT))

            # --- persistent per-batch buffers ---
            q_all = const.tile([P, NT, CQ], bf16)       # biased Q^T
            kn_all = const.tile([P, NT, CQ + 1], bf16)  # [Kn^T | 1]
            # 66-wide slices (fp32r matmul moving size must be even);
            # carved from the DMA-zeroed f32r const so pad columns are 0
            kn_red = ckr_sb[:, 129:327].rearrange("p (h m) -> p h m", h=3)
            qs_all = const.tile([P, NT, CQ + 1], f32r)  # [s*Q^T | gamma*tailor]
            q_ss = const.tile([P, NT], f32)
            qd = const.tile([P, NT], f32)
            qsq = const.tile([P, NT, CQ], bf16)         # scratch squares/prods
            qf_all = const.tile([CQ + 1, N], f32r)      # transposed qs
            at_sb = const.tile([P, KT, CQ + 1], bf16)   # A_aug^T slices
            m1_sb = const.tile([CQ + 1, C], f32r)
            ks_eps = const.tile([P, CQ], f32)
            ksrow = const.tile([1, CQ + 1], f32r)
            M = CQ + 1

            # aug ones column for all kn tiles in one strided write
            nc.vector.tensor_copy(out=kn_all[:, :, CQ], in_=ones32_sb)
            # preload activation tables (Square/Sqrt) while DMAs stream
            warm = const.tile([1, 1], f32)
            nc.scalar.activation(out=warm, in_=wqk_sb[0:1, 0, 0:1],
                                 func=mybir.ActivationFunctionType.Square)
            nc.scalar.activation(out=warm, in_=warm,
                                 func=mybir.ActivationFunctionType.Sqrt)

            with tc.tile_pool(name="psQK", bufs=4, space="PSUM") as psQKp, \
                 tc.tile_pool(name="psXT", bufs=3, space="PSUM") as psXTp, \
                 tc.tile_pool(name="psA", bufs=1, space="PSUM") as psAp:
                at_ps = psAp.tile([P, KT, M], f32)

                # ---- loop 1: stream x; QK projection + x^T + kn + A ------
                pending = []
                xts = {}
                xa_g = None
                x_base = 0
                xc = 0
                for i in range(NT):
                    if sum(XCHUNKS[:xc]) == i:
                        g = XCHUNKS[xc]
                        xa_g = xapool.tile([P, g, KT, P], bf16,
                                           name=f"xa_{xc}", tag="xa")
                        nc.sync.dma_start(
                            out=xa_g,
                            in_=xa_ap[:, i:i + g, :].rearrange(
                                "p g (k n) -> p g k n", k=KT))
                        x_base = i
                        xc += 1
                    xa_t = xa_g[:, i - x_base]

                    psqk = psQKp.tile([P, QK], f32, name=f"qk_{i}", tag="qk")
                    xt_ps = psXTp.tile([P, KT, P], bf16, name=f"xt_{i}",
                                       tag="xt")
                    for k in range(KT):
                        nc.tensor.matmul(psqk, xa_t[:, k, :], wqk_sb[:, k, :],
                                         start=(k == 0), stop=False)
                    # fold Q,K biases in as rank-1 ones^T (x) [bq|bk]
                    nc.tensor.matmul(psqk, ones1p_sb, brow_sb,
                                     start=False, stop=True)
                    # x^T tile via PE transposes (bf16)
                    for k in range(KT):
                        nc.tensor.transpose(xt_ps[:, k, :], xa_t[:, k, :],
                                            identb_sb)
                    xt = xtpool.tile([P, KT, P], bf16, name=f"xts_{i}",
                                     tag="xts")
                    xts[i] = xt
                    nc.vector.tensor_copy(out=xt, in_=xt_ps)

                    # A^T accumulation, ALAG tiles behind
                    if len(pending) >= ALAG:
                        j = pending.pop(0)
                        xtj = xts.pop(j)
                        for k in range(KT):
                            # start zeroes the WHOLE bank: only the very
                            # first matmul into at_ps may set it
                            nc.tensor.matmul(at_ps[:, k, :], xtj[:, k, :],
                                             kn_all[:, j, :],
                                             start=(j == 0 and k == 0),
                                             stop=False,
                                             skip_group_check=True)

                    # K chain: kss -> 1/||K|| -> kn (bf16), straight off PSUM
                    sck = scpool.tile([P, CQ], f32, name=f"sck_{i}", tag="sck")
                    kss = scpool.tile([P, 1], f32, name=f"kss_{i}", tag="kss")
                    nc.scalar.activation(
                        out=sck, in_=psqk[:, CQ:QK],
                        func=mybir.ActivationFunctionType.Square,
                        accum_out=kss)
                    krt = scpool.tile([P, 1], f32, name=f"krt_{i}", tag="krt")
                    nc.scalar.activation(out=krt, in_=kss,
                                         func=mybir.ActivationFunctionType.Sqrt)
                    krs = scpool.tile([P, 1], f32, name=f"krs_{i}", tag="krs")
                    nc.vector.reciprocal(out=krs, in_=krt)
                    # GPSIMD cannot read PSUM: kn scale on DVE, Q copy on Act
                    nc.vector.tensor_scalar_mul(out=kn_all[:, i, 0:CQ],
                                                in0=psqk[:, CQ:QK],
                                                scalar1=krs)
                    nc.scalar.copy(out=q_all[:, i, :], in_=psqk[:, 0:CQ])
                    pending.append(i)
                    # batched q squares + kn partial sums: 0..15 at i=15,
                    # 16..30 at i=30, tile 31 separately (so the mid-phase
                    # barrier chain only waits on tile 31's own tiny ops)
                    if i in (NT // 2 - 1, NT - 2):
                        h = 0 if i < NT // 2 else 1
                        h0 = h * (NT // 2)
                        h1 = (h + 1) * (NT // 2) - h
                        with nc.allow_low_precision(reason="q squares bf16"):
                            nc.vector.tensor_mul(out=qsq[:, h0:h1, :],
                                                 in0=q_all[:, h0:h1, :],
                                                 in1=q_all[:, h0:h1, :])
                        qsh = q_ss[:, h0:h1]
                        qss3 = bass.AP(tensor=qsh.tensor, offset=qsh.offset,
                                       ap=[qsh.ap[0], qsh.ap[1], [1, 1]])
                        nc.vector.reduce_sum(out=qss3, in_=qsq[:, h0:h1, :],
                                             axis=mybir.AxisListType.X)
                        ksl = kn_all[:, h0:h1, :]
                        ksw = bass.AP(tensor=ksl.tensor, offset=ksl.offset,
                                      ap=[ksl.ap[0], ksl.ap[2], ksl.ap[1]])
                        krl = kn_red[:, h, 0:CQ + 1]
                        kr3 = bass.AP(tensor=krl.tensor, offset=krl.offset,
                                      ap=[krl.ap[0], krl.ap[1], [1, 1]])
                        with nc.allow_low_precision(reason="f32r ksum"):
                            nc.vector.reduce_sum(out=kr3, in_=ksw,
                                                 axis=mybir.AxisListType.X)
                    if i == NT - 1:
                        nc.vector.tensor_copy(out=kn_red[:, 2, 0:CQ + 1],
                                              in_=kn_all[:, i, :])
                        scq = scpool.tile([P, CQ], bf16, name="scq31",
                                          tag="scq")
                        with nc.allow_low_precision(reason="scratch sq"):
                            nc.scalar.activation(
                                out=scq, in_=q_all[:, i, :],
                                func=mybir.ActivationFunctionType.Square,
                                accum_out=q_ss[:, i:i + 1])

                # drain A^T accumulation
                for idx, j in enumerate(pending):
                    xtj = xts.pop(j)
                    last = idx == len(pending) - 1
                    for k in range(KT):
                        nc.tensor.matmul(at_ps[:, k, :], xtj[:, k, :],
                                         kn_all[:, j, :],
                                         start=False, stop=last,
                                         skip_group_check=True)
                # A_aug^T -> bf16 sbuf (Act; one strided op)
                nc.scalar.copy(out=at_sb, in_=at_ps)

            with tc.tile_pool(name="psB", bufs=1, space="PSUM") as psB, \
                 tc.tile_pool(name="psT", bufs=2, space="PSUM") as psT_pool:
                # [ksum; N] row: ones^T @ kn_red (3 slices), add slices
                MP = CQ + 2
                ksr_ps = psB.tile([1, 3 * MP], f32)
                nc.tensor.matmul(
                    ksr_ps, onescol_sb,
                    kn_red.rearrange("p h m -> p (h m)"),
                    start=True, stop=True)
                ksr_sb = const.tile([1, 3 * MP], f32r)
                nc.vector.tensor_copy(out=ksr_sb, in_=ksr_ps)
                ksrow2 = const.tile([1, M], f32r)
                nc.vector.tensor_add(out=ksrow2, in0=ksr_sb[0:1, 0:M],
                                     in1=ksr_sb[0:1, MP:MP + M])
                nc.vector.tensor_add(out=ksrow, in0=ksrow2,
                                     in1=ksr_sb[0:1, 2 * MP:2 * MP + M])
                rep_ps = psB.tile([P, CQ], f32)
                nc.tensor.matmul(rep_ps, onesrow_sb, ksrow[0:1, 0:CQ],
                                 start=True, stop=True)
                nc.vector.tensor_scalar_add(out=ks_eps, in0=rep_ps,
                                            scalar1=EPS)
                ksb = const.tile([P, CQ], bf16)
                nc.vector.tensor_copy(out=ksb, in_=ks_eps)

                # M1 = A_aug^T-slices @ WvT + [ksum;N] (x) bv
                m1_ps = psB.tile([M, C], f32)
                for k in range(KT):
                    nc.tensor.matmul(m1_ps, at_sb[:, k, :], wvt_sb[:, k, :],
                                     start=(k == 0), stop=False)
                nc.tensor.matmul(m1_ps, ksrow, bvrow_sb,
                                 start=False, stop=True)
                # evict on Act (GPSIMD cannot read PSUM; DVE busy with qd)
                nc.scalar.copy(out=m1_sb, in_=m1_ps)

                # tailor scalars
                qrt_all = const.tile([P, NT], f32)
                nc.scalar.activation(out=qrt_all, in_=q_ss,
                                     func=mybir.ActivationFunctionType.Sqrt)
                qrs_all = const.tile([P, NT], f32)
                nc.vector.reciprocal(out=qrs_all, in_=qrt_all)
                ksb_ap = bass.AP(tensor=ksb.tensor, offset=ksb.offset,
                                 ap=[ksb.ap[0], [0, NT], [1, CQ]])
                nc.vector.tensor_mul(out=qsq, in0=q_all, in1=ksb_ap)
                nc.vector.reduce_sum(out=qd, in_=qsq,
                                     axis=mybir.AxisListType.X)
                dn = const.tile([P, NT], f32)
                nc.vector.tensor_mul(out=dn, in0=qd, in1=qrs_all)
                nc.vector.tensor_scalar_add(out=dn, in0=dn, scalar1=float(N))
                tailor = const.tile([P, NT], f32)
                nc.vector.reciprocal(out=tailor, in_=dn)
                s_all = const.tile([P, NT], bf16)
                nc.vector.tensor_mul(out=s_all, in0=tailor, in1=qrs_all)
                nc.vector.tensor_copy(out=qs_all[:, :, CQ], in_=tailor)

                NG = NCHUNK // P  # 4 tiles per group
                NGR = NT // NG
                # qs scale muls, alternating DVE/Pool
                for g in range(NGR):
                    s_sl = s_all[:, g * NG:(g + 1) * NG]
                    eng = nc.vector if g % 2 == 0 else nc.gpsimd
                    eng.tensor_mul(
                        out=qs_all[:, g * NG:(g + 1) * NG, 0:CQ],
                        in0=q_all[:, g * NG:(g + 1) * NG, :],
                        in1=bass.AP(tensor=s_sl.tensor, offset=s_sl.offset,
                                    ap=[s_sl.ap[0], s_sl.ap[1], [0, CQ]]))
                # all transposes + qf evictions (Act/DVE alternate)
                for g in range(NGR):
                    tr_ps = psT_pool.tile([CQ + 1, NG, P], f32r,
                                          name=f"tr_{g}", tag="tr")
                    for u in range(NG):
                        nc.tensor.transpose(tr_ps[:, u, :],
                                            qs_all[:, g * NG + u, :],
                                            identf_sb)
                    dstq = qf_all[:, g * NCHUNK:(g + 1) * NCHUNK]
                    srcq = tr_ps.rearrange("m u n -> m (u n)")
                    if g % 2 == 0:
                        nc.scalar.copy(out=dstq, in_=srcq)
                    else:
                        nc.vector.tensor_copy(out=dstq, in_=srcq)

            # ---- out pass: matmuls, pair evictions, one DMA/group --------
            with tc.tile_pool(name="psE", bufs=4, space="PSUM") as psE_pool:
                for g in range(NT // NG):
                    dst = obufp.tile([P, KT, NCHUNK], bf16,
                                     name=f"ob_{g}", tag="ob")
                    for half in range(2):
                        out_ps = psE_pool.tile([P, 2, NCHUNK], f32,
                                               name=f"ops_{half}_{g}",
                                               tag="ops")
                        for ci in range(2):
                            c = half * 2 + ci
                            nc.tensor.matmul(
                                out_ps[:, ci, :],
                                m1_sb[:, c * P:(c + 1) * P],
                                qf_all[:, g * NCHUNK:(g + 1) * NCHUNK],
                                start=True, stop=True)
                        dsth = dst[:, half * 2:half * 2 + 2, :]
                        if half == 0:
                            nc.scalar.copy(out=dsth, in_=out_ps)
                        else:
                            nc.vector.tensor_copy(out=dsth, in_=out_ps)
                    nc.sync.dma_start(
                        out=out_d.ap()[:, g * NCHUNK:(g + 1) * NCHUNK]
                        .rearrange("(c p) n -> p c n", p=P),
                        in_=dst)

    nc.compile()
    return nc


def _get_nc():
    if "nc" not in _CACHE:
        _CACHE["nc"] = _build()
    return _CACHE["nc"]


def _prep_inputs(x, Wq, bq, Wk, bk, Wv, bv, gamma):
    import ml_dtypes
    bf = ml_dtypes.bfloat16
    x = np.ascontiguousarray(np.asarray(x, dtype=np.float32)).reshape(B, C, N)
    # XA[b, p, i, k*128+j] = x[b, k*128+p, i*128+j]
    xa = np.ascontiguousarray(
        x.reshape(B, KT, P, NT, P).transpose(0, 2, 3, 1, 4)
        .reshape(B, P, NT, C).astype(bf))
    wqk = np.ascontiguousarray(np.concatenate(
        [np.asarray(Wq, np.float32).T,
         np.asarray(Wk, np.float32).T], axis=1).astype(bf))
    # wvt[p, k, c] = gamma * Wv.T[k*128+p, c]  (gamma folded in)
    gscal = np.float32(np.asarray(gamma).reshape(-1)[0])
    wvt = np.ascontiguousarray(
        (np.asarray(Wv, np.float32).T * gscal).reshape(KT, P, C)
        .transpose(1, 0, 2).reshape(P, KT * C).astype(bf))
    ckr = np.concatenate([
        np.eye(P, dtype=np.float32),                             # identf
        np.ones((P, 1), dtype=np.float32),                       # onescol
        np.zeros((P, 198), dtype=np.float32),                    # kn_red init
    ], axis=1)
    cprb = np.concatenate([
        np.concatenate([np.asarray(bq, np.float32),
                        np.asarray(bk, np.float32)])[None, :],   # brow
        np.ones((1, P), dtype=np.float32),                       # ones1p
    ], axis=1).astype(bf)
    cprr = np.concatenate([
        (np.asarray(bv, np.float32) * gscal)[None, :],           # gamma*bv
        np.ones((1, P), dtype=np.float32),                       # onesrow
    ], axis=1)
    return {
        "xa": xa,
        "wqk": wqk,
        "identb": np.concatenate(
            [np.eye(P, dtype=np.float32),
             np.ones((P, NT), dtype=np.float32)], axis=1).astype(bf),
        "cprb": np.ascontiguousarray(cprb),
        "ckr": np.ascontiguousarray(ckr),
        "cprr": np.ascontiguousarray(cprr),
        "wvt": wvt,
    }


def kernel(x, Wq, bq, Wk, bk, Wv, bv, gamma, _trace=False):
    from concourse.bass_utils import run_bass_kernel_spmd

    common = _prep_inputs(x, Wq, bq, Wk, bk, Wv, bv, gamma)
    xa = common.pop("xa")
    nc = _get_nc()
    in_maps = [{"xa": xa[i], **common} for i in range(B)]
    res = run_bass_kernel_spmd(nc, in_maps, list(range(B)), trace=_trace)
    out = np.stack([np.asarray(res.results[i]["out"]).astype(np.float32)
                    for i in range(B)])
    if _trace:
        _CACHE["last_results"] = res
    return out.reshape(B, C, H, W)
